# revision 1
# baseline (speedup 1.0000x reference)
"""Trainium2 Bass kernel for nn_MicroBiMambaBackbone (v2).

Data-parallel over batch (B=8 -> 8 cores, 1 sample/core).
Changes vs v1 baseline:
  - delta/u replication for the 16 scan tiles now goes through a DRAM
    round-trip DMA broadcast (stride-0 partition reads) instead of PE
    selector matmuls: kills ~350us of PE matmul time.
  - dA exp reads the replicated delta straight from SBUF (scalar engine).
  - LayerNorm stats via PE matmuls (I - J/64 and J/64 weights) instead of
    gpsimd partition_all_reduce.
  - softplus(dt) via the native Softplus activation table.
  - D_skip*xc folded into the yacc PSUM accumulation as diagonal matmuls.
  - dBx/pmult run as plain bf16 tensor_tensor ops balanced between the
    vector and gpsimd engines (scan itself only exists on vector).
  - hf/hb layer fronts interleaved into the other chain's scan phase.
"""

import sys

sys.path.insert(0, "/opt/trn_rl_repo")

from contextlib import ExitStack

import ml_dtypes
import numpy as np

import concourse.bacc as bacc
import concourse.bass as bass
import concourse.mybir as mybir
import concourse.tile as tile

BF = mybir.dt.bfloat16
F32 = mybir.dt.float32
AF = mybir.ActivationFunctionType
OP = mybir.AluOpType

B, L_FULL, IN_DIM = 8, 2048, 5
D_MODEL, OUT_DIM = 64, 64
N_LAYERS, D_INNER, N_STATE, DT_RANK, K = 2, 128, 16, 4, 4
T = 2 * N_LAYERS
N_CORES = 8

MM_F = 512  # max matmul free dim (one PSUM bank of f32)

# engine assignment for the per-state elementwise multiplies
DBX_G = set()               # gpsimd tt poisons concurrent DVE ops (shared SBUF ports)
PM_G = set()
MID_STATES = (3, 6, 9, 12)  # scan states after which front phases issue


def _mm(nc, out, lhsT, rhs, start=True, stop=True):
    F = rhs.shape[-1]
    for j in range(0, F, MM_F):
        e = min(j + MM_F, F)
        nc.tensor.matmul(out[:, j:e], lhsT, rhs[:, j:e], start=start, stop=stop)


def build_nc(L=L_FULL):
    nc = bacc.Bacc("TRN2", target_bir_lowering=False)
    H = L // 2

    # ---------------- DRAM I/O ----------------
    d_xT = nc.dram_tensor("xT", (IN_DIM, L), BF, kind="ExternalInput")
    d_Wemb = nc.dram_tensor("Wemb", (IN_DIM, D_MODEL), BF, kind="ExternalInput")
    d_bemb = nc.dram_tensor("bemb", (D_MODEL, 1), F32, kind="ExternalInput")
    d_peT = nc.dram_tensor("peT", (D_MODEL, L), BF, kind="ExternalInput")
    d_Wstat = nc.dram_tensor("Wstat", (D_MODEL, 2 * D_MODEL), BF, kind="ExternalInput")
    d_Win = nc.dram_tensor("Win", (D_MODEL, T * 2 * D_INNER), BF, kind="ExternalInput")
    d_beta = nc.dram_tensor("beta", (D_INNER, 2 * T), F32, kind="ExternalInput")
    d_cdiag = nc.dram_tensor("cdiag", (D_INNER, T * K * D_INNER), BF, kind="ExternalInput")
    d_bconv = nc.dram_tensor("bconv", (D_INNER, T), F32, kind="ExternalInput")
    d_Wxdt = nc.dram_tensor("Wxdt", (D_INNER, T * DT_RANK), BF, kind="ExternalInput")
    d_Wdt = nc.dram_tensor("Wdt", (DT_RANK, T * D_INNER), BF, kind="ExternalInput")
    d_bdt = nc.dram_tensor("bdt", (D_INNER, T), F32, kind="ExternalInput")
    d_WxB = nc.dram_tensor("WxB", (D_INNER, T * D_INNER), BF, kind="ExternalInput")
    d_WxC = nc.dram_tensor("WxC", (D_INNER, T * D_INNER), BF, kind="ExternalInput")
    d_sum8 = nc.dram_tensor("sum8", (D_INNER, 8 * 64), BF, kind="ExternalInput")
    d_dskd = nc.dram_tensor("dskd", (D_INNER, T * D_INNER), BF, kind="ExternalInput")
    d_Acol = nc.dram_tensor("Acol", (D_INNER, T * N_STATE), F32, kind="ExternalInput")
    d_Wout = nc.dram_tensor("Wout", (D_INNER, T * D_MODEL), BF, kind="ExternalInput")
    d_Wproj = nc.dram_tensor("Wproj", (2 * D_MODEL, OUT_DIM), F32, kind="ExternalInput")
    d_bproj = nc.dram_tensor("bproj", (OUT_DIM, 1), F32, kind="ExternalInput")
    d_out = nc.dram_tensor("out", (OUT_DIM, 1), F32, kind="ExternalOutput")

    with ExitStack() as ctx:
        tc = ctx.enter_context(tile.TileContext(nc))
        wp = ctx.enter_context(tc.tile_pool(name="weights", bufs=1))
        hp = ctx.enter_context(tc.tile_pool(name="hres", bufs=2))
        ap = ctx.enter_context(tc.tile_pool(name="acts", bufs=1))
        sp = ctx.enter_context(tc.tile_pool(name="scan", bufs=2))
        spr = ctx.enter_context(tc.tile_pool(name="rep", bufs=3))
        dp = ctx.enter_context(tc.tile_pool(name="dscr", bufs=1, space="DRAM"))
        psf = ctx.enter_context(tc.tile_pool(name="psF", bufs=1, space="PSUM"))
        psy = ctx.enter_context(tc.tile_pool(name="psY", bufs=1, space="PSUM"))

        # ---------------- load weights ----------------
        def wload(d, shape, dtype, nsplit=1):
            t = wp.tile(list(shape), dtype, tag="w_" + d.name)
            f = shape[1]
            step = (f + nsplit - 1) // nsplit
            for j in range(0, f, step):
                e = min(j + step, f)
                nc.sync.dma_start(t[:, j:e], d[:, j:e])
            return t

        s_Wemb = wload(d_Wemb, (IN_DIM, D_MODEL), BF)
        s_bemb = wload(d_bemb, (D_MODEL, 1), F32)
        s_Wstat = wload(d_Wstat, (D_MODEL, 2 * D_MODEL), BF)
        s_Win = wload(d_Win, (D_MODEL, T * 2 * D_INNER), BF)
        s_beta = wload(d_beta, (D_INNER, 2 * T), F32)
        s_cdiag = wload(d_cdiag, (D_INNER, T * K * D_INNER), BF, nsplit=2)
        s_bconv = wload(d_bconv, (D_INNER, T), F32)
        s_Wxdt = wload(d_Wxdt, (D_INNER, T * DT_RANK), BF)
        s_Wdt = wload(d_Wdt, (DT_RANK, T * D_INNER), BF)
        s_bdt = wload(d_bdt, (D_INNER, T), F32)
        s_WxB = wload(d_WxB, (D_INNER, T * D_INNER), BF)
        s_WxC = wload(d_WxC, (D_INNER, T * D_INNER), BF)
        s_Acol = wload(d_Acol, (D_INNER, T * N_STATE), F32)
        s_sum8 = wload(d_sum8, (D_INNER, 8 * 64), BF)
        s_dskd = wload(d_dskd, (D_INNER, T * D_INNER), BF)
        s_Wout = wload(d_Wout, (D_INNER, T * D_MODEL), BF)
        s_Wproj = wload(d_Wproj, (2 * D_MODEL, OUT_DIM), F32)
        s_bproj = wload(d_bproj, (OUT_DIM, 1), F32)
        s_eps = wp.tile([D_MODEL, 1], F32, tag="eps")
        nc.vector.memset(s_eps[:], 1e-5)

        # ---------------- embedding ----------------
        with tc.tile_pool(name="embin", bufs=1) as ep:
            s_xT = ep.tile([IN_DIM, L], BF, tag="xT")
            nc.sync.dma_start(s_xT[:], d_xT[:])
            s_peT = ep.tile([D_MODEL, L], BF, tag="peT")
            for j in (0, H):
                nc.sync.dma_start(s_peT[:, j:j + H], d_peT[:, j:j + H])
            h_f = hp.tile([D_MODEL, L], BF, tag="hf")
            for j in (0, H):
                eP = psf.tile([D_INNER, H], F32, tag="mmA")
                _mm(nc, eP[0:D_MODEL, :], s_Wemb[:], s_xT[:, j:j + H])
                nc.vector.scalar_tensor_tensor(
                    h_f[:, j:j + H], eP[0:D_MODEL, :], s_bemb[:], s_peT[:, j:j + H],
                    OP.add, OP.add)
            h_b = hp.tile([D_MODEL, L], BF, tag="hb")
            nc.scalar.activation(h_b[:], h_f[:, ::-1], AF.Identity)

        # ---------------- one mamba layer ----------------
        def front_phases(l, c, h_in):
            """LN + in-proj + conv + dt + u + bm/cm + scratch writes."""
            t = {}
            # --- LN: hm = (I - J/64) h ; var = J/64 hm^2 ---
            hmb = ap.tile([D_MODEL, L], BF, tag="hmb")
            inv = ap.tile([D_MODEL, L], BF, tag="inv")
            hmP = [psf.tile([D_INNER, H], F32, tag=tg, name="hmP" + tg) for tg in ("mmA", "mmB")]
            sq = [ap.tile([D_MODEL, H], BF, tag=tg, name="t" + tg) for tg in ("sq0", "sq1")]
            for i, j in enumerate((0, H)):
                _mm(nc, hmP[i][0:D_MODEL, :], s_Wstat[:, 0:D_MODEL], h_in[:, j:j + H])
            for i in (0, 1):
                nc.scalar.activation(sq[i][:], hmP[i][0:D_MODEL, :], AF.Square)
            for i, j in enumerate((0, H)):
                nc.scalar.activation(hmb[:, j:j + H], hmP[i][0:D_MODEL, :], AF.Identity)
            varP = [psf.tile([D_INNER, H], F32, tag=tg, name="varP" + tg) for tg in ("mmA", "mmB")]
            for i in (0, 1):
                _mm(nc, varP[i][0:D_MODEL, :], s_Wstat[:, D_MODEL:2 * D_MODEL], sq[i][:])
            for i, j in enumerate((0, H)):
                nc.scalar.activation(inv[:, j:j + H], varP[i][0:D_MODEL, :],
                                     AF.Abs_reciprocal_sqrt, bias=s_eps[:])
            hn = ap.tile([D_MODEL, L], BF, tag="hn")
            nc.vector.tensor_tensor(hn[:], hmb[:], inv[:], OP.mult)
            yield t

            # --- in-proj (xi into padded conv input, z -> silu) ---
            xi = ap.tile([D_INNER, L + K - 1], BF, tag="xi")
            nc.vector.memset(xi[:, 0:K - 1], 0.0)
            sz = ap.tile([D_INNER, L], BF, tag="sz" + c)
            w_in = s_Win[:, l * 2 * D_INNER:(l + 1) * 2 * D_INNER]
            for j in (0, H):
                xiP = psf.tile([D_INNER, H], F32, tag="mmA")
                _mm(nc, xiP, w_in[:, 0:D_INNER], hn[:, j:j + H])
                nc.scalar.activation(xi[:, K - 1 + j:K - 1 + j + H], xiP[:],
                                     AF.Identity, bias=s_beta[:, 2 * l:2 * l + 1])
                zP = psf.tile([D_INNER, H], F32, tag="mmB")
                _mm(nc, zP, w_in[:, D_INNER:2 * D_INNER], hn[:, j:j + H])
                nc.scalar.activation(sz[:, j:j + H], zP[:], AF.Silu,
                                     bias=s_beta[:, 2 * l + 1:2 * l + 2])
            # --- causal depthwise conv (4 diag matmuls) + silu ---
            xc = ap.tile([D_INNER, L], BF, tag="xc" + c)
            for j in (0, H):
                cP = psf.tile([D_INNER, H], F32, tag="mmA")
                for k in range(K):
                    dg = s_cdiag[:, (l * K + k) * D_INNER:(l * K + k + 1) * D_INNER]
                    _mm(nc, cP, dg, xi[:, j + k:j + k + H],
                        start=(k == 0), stop=(k == K - 1))
                nc.scalar.activation(xc[:, j:j + H], cP[:], AF.Silu,
                                     bias=s_bconv[:, l:l + 1])
            yield t
            # --- dt path -> delta = softplus(dt @ Wdt + b_dt) ---
            dt_bf = ap.tile([DT_RANK, L], BF, tag="dtbf")
            for j in (0, H):
                dtP = psf.tile([D_INNER, H], F32, tag="mmB")
                _mm(nc, dtP[0:DT_RANK, :], s_Wxdt[:, l * DT_RANK:(l + 1) * DT_RANK],
                    xc[:, j:j + H])
                nc.scalar.activation(dt_bf[:, j:j + H], dtP[0:DT_RANK, :], AF.Identity)
            delta = ap.tile([D_INNER, L], BF, tag="delta")
            dpP = [psf.tile([D_INNER, H], F32, tag=tg, name="dpP" + tg) for tg in ("mmA", "mmB")]
            dexp = [ap.tile([D_INNER, H], BF, tag=tg, name="t" + tg) for tg in ("dexp0", "dexp1")]
            for i, j in enumerate((0, H)):
                _mm(nc, dpP[i], s_Wdt[:, l * D_INNER:(l + 1) * D_INNER],
                    dt_bf[:, j:j + H])
            for i in (0, 1):
                nc.scalar.activation(dexp[i][:], dpP[i][:], AF.Exp,
                                     bias=s_bdt[:, l:l + 1])
            for i, j in enumerate((0, H)):
                nc.scalar.activation(delta[:, j:j + H], dexp[i][:], AF.Ln, bias=1.0)
            # --- u = delta * xc ---
            u = ap.tile([D_INNER, L], BF, tag="u")
            nc.vector.tensor_tensor(u[:], delta[:], xc[:], OP.mult)
            yield t
            # --- bm/cm replicated coefficient tiles ---
            for nm, w_all in (("bm", s_WxB), ("cm", s_WxC)):
                rb = ap.tile([D_INNER, L], BF, tag=nm + c)
                for j in (0, H):
                    rP = psf.tile([D_INNER, H], F32, tag="mmB")
                    _mm(nc, rP, w_all[:, l * D_INNER:(l + 1) * D_INNER], xc[:, j:j + H])
                    nc.scalar.activation(rb[:, j:j + H], rP[:], AF.Identity)
                t[nm] = rb
            # --- scratch DRAM writes for replication ---
            scrD = dp.tile([D_INNER, L], BF, tag="scrD" + c)
            scrU = dp.tile([D_INNER, L], BF, tag="scrU" + c)
            for j in (0, H):
                nc.sync.dma_start(scrD[:, j:j + H], delta[:, j:j + H])
                nc.sync.dma_start(scrU[:, j:j + H], u[:, j:j + H])
            t.update(xc=xc, sz=sz, scrD=scrD, scrU=scrU, h_in=h_in, l=l, c=c)
            yield t

        def scan_post(t, mid_cb=None):
            l, c = t["l"], t["c"]
            yaccP = psy.tile([D_INNER, L], F32, tag="yacc")
            # D_skip * xc seeds the psum accumulation (diag matmuls per block)
            for blk in (0, 64):
                dgw = s_dskd[:, l * D_INNER + blk:l * D_INNER + blk + 64]
                for j in range(0, L, MM_F):
                    nc.tensor.matmul(yaccP[blk:blk + 64, j:j + MM_F], dgw,
                                     t["xc"][:, j:j + MM_F], start=True, stop=False,
                                     skip_group_check=True)
            for s in range(N_STATE):
                dl = spr.tile([D_INNER, L], BF, tag="dl")
                nc.sync.dma_start(
                    dl[:], t["scrD"][8 * s:8 * s + 8, :].unsqueeze(1)
                    .broadcast_to((8, N_STATE, L)))
                ur = spr.tile([D_INNER, L], BF, tag="ur")
                nc.sync.dma_start(
                    ur[:], t["scrU"][8 * s:8 * s + 8, :].unsqueeze(1)
                    .broadcast_to((8, N_STATE, L)))
                dA = sp.tile([D_INNER, L], BF, tag="dA")
                nc.scalar.activation(dA[:], dl[:], AF.Exp,
                                     scale=s_Acol[:, l * N_STATE + s:l * N_STATE + s + 1])
                dBx = sp.tile([D_INNER, L], BF, tag="dBx")
                eng = nc.gpsimd if s in DBX_G else nc.vector
                eng.tensor_tensor(dBx[:], ur[:], t["bm"][:], OP.mult)
                hs = sp.tile([D_INNER, L], BF, tag="hs")
                nc.vector.tensor_tensor_scan(hs[:, 0:H], dA[:, 0:H], dBx[:, 0:H],
                                             0.0, OP.mult, OP.add)
                nc.vector.tensor_tensor_scan(hs[:, H:L], dA[:, H:L], dBx[:, H:L],
                                             hs[:, H - 1:H], OP.mult, OP.add)
                p = sp.tile([D_INNER, L], BF, tag="p")
                eng = nc.gpsimd if s in PM_G else nc.vector
                eng.tensor_tensor(p[:], t["cm"][:], hs[:], OP.mult)
                k = s % 8
                blk = (s // 8) * 64
                for j in range(0, L, MM_F):
                    nc.tensor.matmul(yaccP[blk:blk + 64, j:j + MM_F],
                                     s_sum8[:, k * 64:(k + 1) * 64], p[:, j:j + MM_F],
                                     start=False, stop=(k == 7),
                                     skip_group_check=True)
                if mid_cb is not None and s in MID_STATES:
                    mid_cb()
            # --- postprocess: y = yacc (has D_skip term), gate, out-proj ---
            yaccS = ap.tile([D_INNER, L], BF, tag="yac" + c)
            for j in (0, H):
                nc.scalar.activation(yaccS[:, j:j + H], yaccP[:, j:j + H], AF.Identity)
            yg = ap.tile([D_INNER, L], BF, tag="yg" + c)
            nc.vector.tensor_tensor(yg[:], yaccS[:], t["sz"], OP.mult)
            o_s = ap.tile([D_MODEL, L], BF, tag="os" + c)
            for j in (0, H):
                oP = psf.tile([D_INNER, H], F32, tag="mmA")
                _mm(nc, oP[0:D_MODEL, :], s_Wout[:, l * D_MODEL:(l + 1) * D_MODEL],
                    yg[:, j:j + H])
                nc.scalar.activation(o_s[:, j:j + H], oP[0:D_MODEL, :], AF.Identity)
            h_out = hp.tile([D_MODEL, L], BF, tag="h" + c)
            nc.vector.tensor_tensor(h_out[:], t["h_in"][:], o_s[:], OP.add)
            return h_out

        # ---------------- run the 4 layer-units, fronts interleaved ----------
        units = [(0, "f"), (N_LAYERS, "b"), (1, "f"), (N_LAYERS + 1, "b")]
        hcur = {"f": h_f, "b": h_b}

        def run_all(gen):
            t = None
            for t in gen:
                pass
            return t

        ctx_next = run_all(front_phases(*units[0], hcur[units[0][1]]))
        for i in range(len(units)):
            t_cur = ctx_next
            holder = {}

            def mk_cb(idx, holder):
                if idx + 1 >= len(units):
                    return None
                ln, cn = units[idx + 1]
                gen = front_phases(ln, cn, hcur[cn])

                def cb():
                    try:
                        holder["ctx"] = next(gen)
                    except StopIteration:
                        pass
                return cb

            h_new = scan_post(t_cur, mk_cb(i, holder))
            hcur[t_cur["c"]] = h_new
            ctx_next = holder.get("ctx")

        # ---------------- head ----------------
        mf = ap.tile([D_MODEL, 1], F32, tag="mf")
        nc.vector.tensor_reduce(mf[:], hcur["f"][:], axis=mybir.AxisListType.X,
                                op=OP.add)
        mb = ap.tile([D_MODEL, 1], F32, tag="mb")
        nc.vector.tensor_reduce(mb[:], hcur["b"][:], axis=mybir.AxisListType.X,
                                op=OP.add)
        zv = ap.tile([2 * D_MODEL, 1], F32, tag="zv")
        nc.sync.dma_start(zv[0:D_MODEL, :], mf[:])
        nc.sync.dma_start(zv[D_MODEL:2 * D_MODEL, :], mb[:])
        oP = psf.tile([D_INNER, 1], F32, tag="mmB")
        nc.tensor.matmul(oP[0:OUT_DIM, :], s_Wproj[:], zv[:])
        ofin = ap.tile([OUT_DIM, 1], F32, tag="ofin")
        nc.scalar.activation(ofin[:], oP[0:OUT_DIM, :], AF.Identity,
                             bias=s_bproj[:])
        nc.sync.dma_start(d_out[:], ofin[:])

    return nc


def prep_inputs(inputs, L=L_FULL):
    bf = ml_dtypes.bfloat16
    f32 = np.float32
    g = {k: np.asarray(v) for k, v in inputs.items()}
    W_in, W_conv, W_x, W_dt = g["W_in"], g["W_conv"], g["W_x"], g["W_dt"]
    ln_w, ln_b = g["ln_w"], g["ln_b"]

    Win = np.concatenate([W_in[l] * ln_w[l][:, None] for l in range(T)], axis=1)
    beta = np.stack([ln_b[l] @ W_in[l] for l in range(T)], 0)
    beta_blob = np.zeros((D_INNER, 2 * T), f32)
    for l in range(T):
        beta_blob[:, 2 * l] = beta[l, :D_INNER]
        beta_blob[:, 2 * l + 1] = beta[l, D_INNER:]
    cdiag = np.zeros((D_INNER, T * K * D_INNER), f32)
    for l in range(T):
        for k in range(K):
            blk = (l * K + k) * D_INNER
            cdiag[np.arange(D_INNER), blk + np.arange(D_INNER)] = W_conv[l, :, 0, k]
    Wxdt = np.concatenate([W_x[l][:, :DT_RANK] for l in range(T)], axis=1)
    Wdt = np.concatenate([W_dt[l] for l in range(T)], axis=1)
    WxB = np.concatenate(
        [np.tile(W_x[l][:, DT_RANK:DT_RANK + N_STATE], (1, 8)) for l in range(T)],
        axis=1)
    WxC = np.concatenate(
        [np.tile(W_x[l][:, DT_RANK + N_STATE:], (1, 8)) for l in range(T)], axis=1)
    sum8 = np.zeros((D_INNER, 8 * 64), f32)
    for k in range(8):
        for gg in range(8):
            sum8[gg * 16:(gg + 1) * 16, k * 64 + k * 8 + gg] = 1.0
    # diag(D_skip) blocks for the yacc seed matmuls
    dskd = np.zeros((D_INNER, T * D_INNER), f32)
    for l in range(T):
        for blk in (0, 64):
            for q in range(64):
                dskd[blk + q, l * D_INNER + blk + q] = g["D_skip"][l][blk + q]
    A = -np.exp(g["A_log"])
    Acol = np.zeros((D_INNER, T * N_STATE), f32)
    for l in range(T):
        for s in range(N_STATE):
            Acol[:, l * N_STATE + s] = A[l][8 * s:8 * s + 8, :].reshape(-1)
    Wout = np.concatenate([g["W_out"][l] for l in range(T)], axis=1)
    # LN stats weights: [I - J/64 | J/64]
    Wstat = np.zeros((D_MODEL, 2 * D_MODEL), f32)
    Wstat[:, 0:D_MODEL] = np.eye(D_MODEL) - 1.0 / D_MODEL
    Wstat[:, D_MODEL:] = 1.0 / D_MODEL

    shared = {
        "Wemb": g["W_emb"].astype(bf),
        "bemb": g["b_emb"].reshape(D_MODEL, 1).astype(f32),
        "peT": np.ascontiguousarray(g["pe"][:L].T).astype(bf),
        "Wstat": Wstat.astype(bf),
        "Win": Win.astype(bf),
        "beta": beta_blob,
        "cdiag": cdiag.astype(bf),
        "bconv": np.ascontiguousarray(g["b_conv"].T).astype(f32),
        "Wxdt": Wxdt.astype(bf),
        "Wdt": Wdt.astype(bf),
        "bdt": np.ascontiguousarray(g["b_dt"].T).astype(f32),
        "WxB": WxB.astype(bf),
        "WxC": WxC.astype(bf),
        "sum8": sum8.astype(bf),
        "dskd": dskd.astype(bf),
        "Acol": Acol.astype(f32),
        "Wout": Wout.astype(bf),
        "Wproj": (g["W_proj"] / L).astype(f32),
        "bproj": g["b_proj"].reshape(OUT_DIM, 1).astype(f32),
    }
    in_maps = []
    for c in range(B):
        m = dict(shared)
        m["xT"] = np.ascontiguousarray(g["x"][c, :L].T).astype(bf)
        in_maps.append(m)
    return in_maps


_CACHE = {}


def kernel(**inputs):
    if "nc" not in _CACHE:
        _CACHE["nc"] = build_nc()
        _CACHE["nc"].finalize()
    nc = _CACHE["nc"]
    in_maps = prep_inputs(inputs)
    from concourse.bass_utils import run_bass_kernel_spmd
    res = run_bass_kernel_spmd(nc, in_maps, core_ids=list(range(N_CORES)))
    out = np.stack([np.asarray(res.results[c]["out"]).reshape(OUT_DIM)
                    for c in range(N_CORES)], axis=0)
    return out.astype(np.float32)

